# revision 6
# baseline (speedup 1.0000x reference)
"""Self-contained Trainium2 Bass kernel for the EdgeNetwork GNN problem.

kernel(**inputs) takes the FULL unsharded inputs and returns the FULL
[100000, 32] output.

Strategy: shard by DESTINATION node range across 8 cores (no collectives).
Host routes each edge to the core owning its dst, sorts by dst, and packs
edges into 512-edge chunks covering <=128 CONSECUTIVE local node ids (a
node's edges never cross a chunk).  Per chunk the device computes

    U^T[(k,j), n] = sum_e S[e, n] * ea[e, k] * x[e, j]      (PE matmuls)
    out[n, i]     = sum_{k,j} U^T[(k,j), n] * B[(k,j), i]   (PE matmuls)

where S[e, n] = 1 iff edge e belongs to the chunk's n-th node.  S is a 0/1
matrix packed on the HOST as fp8e4 (exact) and fed straight to the tensor
engine as the matmul rhs (mixed fp16 x fp8 is supported).  The only
per-edge vector work is the Khatri-Rao product Z[e,(k,j)] = ea[e,k]*x[e,j],
split between the DVE and GPSIMD engines (one broadcast tensor_tensor
each per chunk).  Accumulation happens in fp32 PSUM.

The neighbour gather x = node_attr[src] happens on the HOST (numpy
fancy-indexing) — per-edge x rows are 3.4MB/core in fp16, cheaper to ship
than a 6.4MB node-table replica, and it removes every indirect DMA from
the device program.  Chunk slots map to consecutive node ids, so the
device writes result rows contiguously and the host applies a precomputed
permutation instead of a device-side scatter.
"""

import os
import sys
from contextlib import ExitStack

import numpy as np
import ml_dtypes

for _p in ("/opt/trn_rl_repo", "/root/.axon_site/_ro/trn_rl_repo"):
    if os.path.isdir(_p) and _p not in sys.path:
        sys.path.insert(0, _p)

import concourse.mybir as mybir
import concourse.tile as tile
from concourse import bacc
from concourse.bass_utils import run_bass_kernel_spmd

N_NODES = 100000
D = 32
KE = 16
NCORES = 8
NPC = N_NODES // NCORES
CHUNK = 512          # edges per chunk (4 tiles of 128)
NRUNS = 128          # max consecutive node ids per chunk (S columns)
SUPER = 4096         # edges per superstep (8 chunks)

F32 = mybir.dt.float32
F16 = mybir.dt.float16
F8 = mybir.dt.float8e4
F8NP = ml_dtypes.float8_e4m3

# Z-build tiles built on GPSIMD per chunk: 2 on every 4th chunk, else 1
def _pool_tiles(q):
    return 2 if q % 4 == 0 else 1


# ---------------------------------------------------------------- host prep

def _pack_core_edges(dst_sorted_idx, dst_local):
    """Pack dst-sorted edges into chunks of <=CHUNK edges covering <=NRUNS
    consecutive local node ids.  Every node id occupies exactly one slot of
    exactly one chunk (zero-degree nodes included), so result rows cover
    all nodes.

    Returns (order, slot, node0):
      order [n_chunks*CHUNK] int64: edge id per packed position (-1 = pad)
      slot  [n_chunks*CHUNK] int32: node slot within chunk (NRUNS = pad)
      node0 [n_chunks] int64: first local node id of each chunk
    """
    lengths = np.bincount(dst_local, minlength=NPC).astype(np.int64)
    assert lengths.max(initial=0) <= CHUNK, "single dst exceeds chunk capacity"
    cum = np.concatenate([[0], np.cumsum(lengths)])

    cuts = [0]
    i = 0
    while i < NPC:
        j = min(i + NRUNS, NPC)
        j2 = int(np.searchsorted(cum, cum[i] + CHUNK, side="right")) - 1
        j = min(j, j2)
        assert j > i
        cuts.append(j)
        i = j
    cuts = np.asarray(cuts, dtype=np.int64)
    nch = len(cuts) - 1
    ch_node0 = cuts[:-1]
    ch_e0 = cum[ch_node0]

    n = len(dst_sorted_idx)
    e_chunk = np.searchsorted(ch_e0, np.arange(n), side="right") - 1
    pos = e_chunk * CHUNK + (np.arange(n) - ch_e0[e_chunk])

    order = np.full(nch * CHUNK, -1, np.int64)
    order[pos] = dst_sorted_idx
    slot = np.full(nch * CHUNK, NRUNS, np.int32)
    slot[pos] = dst_local - ch_node0[e_chunk]
    return order, slot, ch_node0


def _prepare(node_attr, edge_attr, pair_indices, kernel, bias):
    dst = np.asarray(pair_indices[:, 0], dtype=np.int64)
    src = np.asarray(pair_indices[:, 1], dtype=np.int64)
    ea = np.asarray(edge_attr, dtype=np.float32)
    kern = np.asarray(kernel, dtype=np.float32)
    bias = np.asarray(bias, dtype=np.float32)

    use_bias = bool(np.any(bias != 0.0))
    if use_bias:
        KP = KE + 1
        kern_full = np.concatenate([kern, bias[None, :]], axis=0)
    else:
        KP = KE
        kern_full = kern
    KG = (KP + 3) // 4
    KPAD = KG * 4

    # B[(k,j), i] = kern[k, i*D + j], zero-padded to KPAD k's
    B = np.zeros((KPAD * D, D), dtype=np.float32)
    Bk = kern_full.reshape(KP, D, D).transpose(0, 2, 1)   # [KP, j, i]
    B[: KP * D] = Bk.reshape(KP * D, D)

    per_core_raw = []
    max_chunks = 0
    for c in range(NCORES):
        lo, hi = c * NPC, (c + 1) * NPC
        sel = np.nonzero((dst >= lo) & (dst < hi))[0]
        d_loc_unsorted = dst[sel] - lo
        s_ord = np.argsort(d_loc_unsorted, kind="stable")
        order, slot, node0 = _pack_core_edges(sel[s_ord],
                                              d_loc_unsorted[s_ord])
        per_core_raw.append((order, slot, node0))
        max_chunks = max(max_chunks, len(node0))

    NSUP = (max_chunks + 7) // 8
    NCH = NSUP * 8
    Epad = NCH * CHUNK

    def swz(a):
        # [NSUP*8*4*128, ...] -> [NSUP, 128, 8*4, ...] (col = q*4 + t)
        a = a.reshape(NSUP, 8, 4, 128, *a.shape[1:])
        return np.ascontiguousarray(np.moveaxis(a, 3, 1))

    per_core = []
    perms = []
    node_f16 = np.ascontiguousarray(node_attr, dtype=np.float16)
    for c in range(NCORES):
        order, slot, node0 = per_core_raw[c]
        nch = len(node0)
        order = np.concatenate([order, np.full((NCH - nch) * CHUNK, -1,
                                               np.int64)])
        slot = np.concatenate([slot, np.full((NCH - nch) * CHUNK, NRUNS,
                                             np.int32)])

        real = order >= 0
        oe = np.where(real, order, 0)

        eaP = np.zeros((Epad, KPAD), dtype=np.float16)
        eaP[real, :KE] = ea[oe[real]].astype(np.float16)
        if use_bias:
            eaP[real, KE] = 1.0
        xP = np.zeros((Epad, D), dtype=np.float16)
        xP[real] = node_f16[src[oe[real]]]

        SP = np.zeros((Epad, NRUNS), dtype=F8NP)
        epos = np.flatnonzero(slot < NRUNS)
        SP[epos, slot[epos]] = 1.0

        # node n of chunk ch=(s,q), slot p -> result row ((s*128 + p)*8 + q)
        cnt = np.diff(np.concatenate([node0, [NPC]]))
        ch_of = np.repeat(np.arange(nch), cnt)
        p_of = np.arange(NPC) - np.repeat(node0, cnt)
        perm = ((ch_of // 8) * 128 + p_of) * 8 + (ch_of % 8)
        perms.append(perm)

        per_core.append(dict(
            ea_sw=swz(eaP).reshape(NSUP, 128, 32 * KPAD),
            x_sw=swz(xP).reshape(NSUP, 128, 32 * D),
            s_sw=swz(SP).reshape(NSUP, 128, 32 * NRUNS),
            B=B.astype(np.float16),
        ))
    meta = dict(Epad=Epad, NSUP=NSUP, KG=KG, KPAD=KPAD, perms=perms)
    return per_core, meta


# ------------------------------------------------------------- bass program

def _build(NSUP, KPAD, KG):
    nc = bacc.Bacc("TRN2", target_bir_lowering=False, debug=False)

    KJ = KPAD * D            # Khatri-Rao width, KG blocks of 128
    ea_d = nc.dram_tensor("ea_sw", [NSUP, 128, 32 * KPAD], F16,
                          kind="ExternalInput").ap()
    x_d = nc.dram_tensor("x_sw", [NSUP, 128, 32 * D], F16,
                         kind="ExternalInput").ap()
    s_d = nc.dram_tensor("s_sw", [NSUP, 128, 32 * NRUNS], F8,
                         kind="ExternalInput").ap()
    b_d = nc.dram_tensor("B", [KJ, D], F16, kind="ExternalInput").ap()
    out_d = nc.dram_tensor("out", [NSUP, 128, 8 * D], F32,
                           kind="ExternalOutput").ap()

    with tile.TileContext(nc) as tc, ExitStack() as ctx:
        const_pool = ctx.enter_context(tc.tile_pool(name="const", bufs=1))
        sup_pool = ctx.enter_context(tc.tile_pool(name="sup", bufs=2))
        z_pool = ctx.enter_context(tc.tile_pool(name="z", bufs=3))
        ut_pool = ctx.enter_context(tc.tile_pool(name="ut", bufs=3))
        ot_pool = ctx.enter_context(tc.tile_pool(name="ot", bufs=2))
        put_pool = ctx.enter_context(
            tc.tile_pool(name="put", bufs=3, space="PSUM"))
        po_pool = ctx.enter_context(
            tc.tile_pool(name="po", bufs=2, space="PSUM"))

        b_sb = const_pool.tile([128, KG * D], F16, tag="b")
        for g in range(KG):
            nc.sync.dma_start(b_sb[:, g * D:(g + 1) * D],
                              b_d[g * 128:(g + 1) * 128, :])

        for s in range(NSUP):
            ea_sb = sup_pool.tile([128, 32 * KPAD], F16, tag="ea")
            nc.sync.dma_start(ea_sb[:], ea_d[s])
            x_sb = sup_pool.tile([128, 32 * D], F16, tag="x")
            nc.sync.dma_start(x_sb[:], x_d[s])
            s_sb = sup_pool.tile([128, 32 * NRUNS], F8, tag="s")
            nc.sync.dma_start(s_sb[:], s_d[s])

            ot = ot_pool.tile([128, 8 * D], F32, tag="ot")

            for q in range(8):
                # Z[e, (t,k,j)] = ea[e, (t,k)] * x[e, (t,j)]
                z_t = z_pool.tile([128, 4 * KJ], F16, tag="z")
                nt = 4 - _pool_tiles(q)

                def zbuild(eng, t0, t1):
                    x_b = x_sb[:, (q * 4 + t0) * D:(q * 4 + t1) * D] \
                        .rearrange("p (t o j) -> p t o j", t=t1 - t0, o=1) \
                        .to_broadcast([128, t1 - t0, KPAD, D])
                    ea_b = ea_sb[:, (q * 4 + t0) * KPAD:(q * 4 + t1) * KPAD] \
                        .rearrange("p (t k o) -> p t k o", t=t1 - t0, o=1) \
                        .to_broadcast([128, t1 - t0, KPAD, D])
                    eng.tensor_tensor(
                        out=z_t[:, t0 * KJ:t1 * KJ]
                            .rearrange("p (t k j) -> p t k j", t=t1 - t0, j=D),
                        in0=x_b, in1=ea_b, op=mybir.AluOpType.mult)

                zbuild(nc.vector, 0, nt)
                zbuild(nc.gpsimd, nt, 4)

                # UT[(kj), n] += Z[e, kj]^T @ S[e, n]  (contract edges)
                ut_ps = put_pool.tile([128, KG * NRUNS], F32, tag="utp")
                for g in range(KG):
                    for t in range(4):
                        nc.tensor.matmul(
                            out=ut_ps[:, g * NRUNS:(g + 1) * NRUNS],
                            lhsT=z_t[:, t * KJ + g * 128:
                                     t * KJ + (g + 1) * 128],
                            rhs=s_sb[:, (q * 4 + t) * NRUNS:
                                     (q * 4 + t + 1) * NRUNS],
                            start=(t == 0), stop=(t == 3))

                ut_sb = ut_pool.tile([128, KG * NRUNS], F16, tag="uts")
                nc.scalar.copy(out=ut_sb[:], in_=ut_ps[:])

                # out[n, i] = sum_g UT_g[kj, n]^T @ B_g[kj, i]
                po = po_pool.tile([128, D], F32, tag="po")
                for g in range(KG):
                    nc.tensor.matmul(
                        out=po[:],
                        lhsT=ut_sb[:, g * NRUNS:(g + 1) * NRUNS],
                        rhs=b_sb[:, g * D:(g + 1) * D],
                        start=(g == 0), stop=(g == KG - 1))
                nc.scalar.copy(out=ot[:, q * D:(q + 1) * D], in_=po[:])

            nc.sync.dma_start(out_d[s], ot[:])

    nc.compile()
    return nc


_CACHE = {}


def kernel(node_attr, edge_attr, pair_indices, kernel, bias):
    per_core, meta = _prepare(node_attr, edge_attr, pair_indices,
                              kernel, bias)
    key = (meta["NSUP"], meta["KPAD"], meta["KG"])
    if key not in _CACHE:
        _CACHE[key] = _build(*key)
    nc = _CACHE[key]
    res = run_bass_kernel_spmd(nc, per_core, list(range(NCORES)))
    out = np.concatenate(
        [res.results[c]["out"].reshape(-1, D)[meta["perms"][c]]
         for c in range(NCORES)], axis=0)
    return np.ascontiguousarray(out, dtype=np.float32)
